# revision 1
# baseline (speedup 1.0000x reference)
"""RWKV block (T=8192, D=2048) on 8 Trainium2 NeuronCores.

Strategy: data-parallel over the sequence (1024 tokens/core) with a
256-token recomputed warmup prefix per core (power-decay attention forgets
at e^{-|w|} per step, |w|>=0.6, so 256 steps of history reproduce the true
WKV state to ~e^{-150} — exact at fp32). Everything runs feature-major
[D partitions, T free]: token-shift is a free-axis offset, LayerNorm stats
are ones-matmuls on the tensor engine, and the WKV recurrence is two
`tensor_tensor_scan` linear scans (state = e^w * state + e^k [*v]) which are
numerically safe unstabilized because k ~ N(0,1). GEMMs run in fp32r
(full-rate on the PE). LN gamma/beta are folded into the following GEMM
weights on the host; weights are host-transposed to [d_in, d_out].
Core 0's warmup block is the wrapped tail x[T-256:], which both feeds the
roll-wrap token shifts and produces rz[T-1] for the FFN branch's wrap row;
a per-core mask input zeroes the scan carry at the warmup/main boundary so
core 0's main block starts from empty state like the reference.
"""
import sys
if '/opt/trn_rl_repo' not in sys.path:
    sys.path.insert(0, '/opt/trn_rl_repo')

from contextlib import ExitStack
import numpy as np

import concourse.bass as bass
import concourse.tile as tile
from concourse import bacc, mybir
from concourse.bass import _add_dep_helper
from concourse.bass_utils import run_bass_kernel_spmd

F32 = mybir.dt.float32
F32R = mybir.dt.float32r
AF = mybir.ActivationFunctionType
OP = mybir.AluOpType

D = 2048
T = 8192
NCORES = 8
TLOC = T // NCORES          # 1024 main tokens per core
H = 256                     # warmup tokens
TBUF = H + TLOC             # 1280
BS = 256                    # token block size
NB = TBUF // BS             # 5 blocks; block 0 is the warmup
NT = D // 128               # 16 partition tiles
JQ = 2                      # j-tiles per psum group (256 output channels)

WNAMES = ['wk', 'wv', 'wr', 'wo', 'wfk', 'wfv', 'wfr']
BNAMES = ['bk', 'bv', 'br', 'bo', 'bfk', 'bfv', 'bfr']
VNAMES = ['mixk', 'mixv', 'mixr', 'fmixk', 'fmixr', 'ew', 'eu', 'cmask']


def build_kernel():
    nc = bacc.Bacc()
    xT = nc.declare_dram_parameter('xT', [D, TBUF], F32R, isOutput=False)
    onescol = nc.declare_dram_parameter('onescol', [128, 1], F32R, isOutput=False)
    onesrow = nc.declare_dram_parameter('onesrow', [1, 128], F32R, isOutput=False)
    wd = {n: nc.declare_dram_parameter(n, [D, D], F32R, isOutput=False)
          for n in WNAMES}
    vd = {n: nc.declare_dram_parameter(n, [D, 1], F32, isOutput=False)
          for n in BNAMES + VNAMES}
    outT = nc.declare_dram_parameter('outT', [D, TLOC], F32, isOutput=True)

    xTt = xT.rearrange('(n p) t -> n p t', p=128)
    outTt = outT.rearrange('(n p) t -> n p t', p=128)
    wdt = {n: w.rearrange('(n p) j -> n p j', p=128) for n, w in wd.items()}
    # per-channel vectors as [128, NT]: col i holds channels [i*128, (i+1)*128)
    vdt = {n: v.rearrange('(n p) o -> p (n o)', p=128) for n, v in vd.items()}

    with tile.TileContext(nc) as tc:
        with ExitStack() as ctx:
            kern(ctx, tc, xTt, wdt, vdt, outTt, onescol, onesrow)
    nc.compile()
    return nc


def kern(ctx, tc, xTt, wdt, vdt, outTt, onescol, onesrow):
    nc = tc.nc

    cons = ctx.enter_context(tc.tile_pool(name='cons', bufs=1))
    cv = {}
    for n in BNAMES + VNAMES:
        cvt = cons.tile([128, NT], F32, tag=f'cv_{n}', name=f'cv_{n}')
        nc.sync.dma_start(cvt[:], vdt[n])
        cv[n] = [cvt[:, i:i + 1] for i in range(NT)]
    ones = cons.tile([128, 1], F32R, tag='ones', name='ones')
    nc.sync.dma_start(ones[:], onescol[:])
    ones_row = cons.tile([1, 128], F32R, tag='ones_row', name='ones_row')
    nc.sync.dma_start(ones_row[:], onesrow[:])

    # persistent boundary-state columns (scan carries / U lead cols)
    colp = ctx.enter_context(tc.tile_pool(name='colp', bufs=1))

    # weight panels [128, JQ*128] per K-tile, double buffered per tag
    wpool = ctx.enter_context(tc.tile_pool(name='wpool', bufs=2))
    # block activation tensors: one tag per (tensor, d-tile), single buffer;
    # Tile recycles a slot as soon as its last reader retires.
    ap = ctx.enter_context(tc.tile_pool(name='ap', bufs=64))     # shared slots
    ap2 = ctx.enter_context(tc.tile_pool(name='ap2', bufs=1))    # long-lived per-i
    wkvp = ctx.enter_context(tc.tile_pool(name='wkvp', bufs=2))  # per-i transients
    scr = ctx.enter_context(tc.tile_pool(name='scr', bufs=2))    # small scratch
    rows = ctx.enter_context(tc.tile_pool(name='rows', bufs=2))  # [1,BS] stats
    psg = ctx.enter_context(tc.tile_pool(name='psg', bufs=6, space='PSUM'))
    pss = ctx.enter_context(tc.tile_pool(name='pss', bufs=1, space='PSUM'))

    def pe_guard(aps):
        """Fused-LDW fp32/fp32r matmuls can carry only ONE sync wait in the
        ISA. Emit a PE NoOp that *reads* the given APs: Tile's dependency
        tracker assigns all cross-engine waits to it through the normal
        wait-clock path, so matmuls ordered behind it on the PE queue
        inherit the observed clocks (waits elided). The APs are stripped
        from the NoOp at lowering (framework-supported sync idiom)."""
        eng = nc.tensor
        inst = mybir.InstNoOp(
            name=nc.get_next_instruction_name(),
            text_hint='pe_guard', bass_nofuse=True,
            ins=[eng.lower_ap(a) for a in aps])
        return eng.add_instruction(inst)

    def gemm(wname, rhs, rhs_insts, out_cb):
        """out[j, :] = sum_d w[d, j] * rhs[d], contraction over all of D.
        rhs: list of 16 fp32r APs [128, BS]. out_cb(jt, psum_ap)."""
        for j0 in range(0, NT, JQ):
            panels = [wpool.tile([128, JQ * 128], F32R, tag=f'w_{kt}', name=f'w_{kt}')
                      for kt in range(NT)]
            for kt in range(NT):
                nc.sync.dma_start(panels[kt][:],
                                  wdt[wname][kt, :, j0 * 128:(j0 + JQ) * 128])
            guard = pe_guard([p[:] for p in panels] + list(rhs))
            for jj in range(JQ):
                pt = psg.tile([128, BS], F32, tag='ps', name='ps')
                for kt in range(NT):
                    mm = nc.tensor.matmul(
                        pt[:], panels[kt][:, jj * 128:(jj + 1) * 128],
                        rhs[kt], start=(kt == 0), stop=(kt == NT - 1))
                    _add_dep_helper(mm.ins, guard.ins, sync=False,
                                    reason='order after guard')
                out_cb(j0 + jj, pt[:])

    def ln_stats(xtiles, xinsts, tagp):
        """Per-token mean/rstd over the partition axis via ones-matmuls.
        xtiles: 16 fp32r APs [128, BS]. Returns (s_b, ms_b) [128, BS]."""
        ps_s = pss.tile([1, BS], F32, tag='st0', name='st0')
        ps_q = pss.tile([1, BS], F32, tag='st1', name='st1')
        sq0 = scr.tile([128, BS], F32R, tag='sq', name='sq', bufs=4)
        nc.scalar.activation(sq0[:], xtiles[0], AF.Square)
        guard = pe_guard(list(xtiles) + [sq0[:], ones[:]])
        for kt in range(NT):
            if kt == 0:
                sq = sq0
            else:
                sq = scr.tile([128, BS], F32R, tag='sq', name='sq', bufs=4)
                nc.scalar.activation(sq[:], xtiles[kt], AF.Square)
            mm = nc.tensor.matmul(ps_s[:], ones[:], xtiles[kt],
                                  start=(kt == 0), stop=(kt == NT - 1))
            _add_dep_helper(mm.ins, guard.ins, sync=False, reason='g')
            mm2 = nc.tensor.matmul(ps_q[:], ones[:], sq[:],
                                   start=(kt == 0), stop=(kt == NT - 1))
            _add_dep_helper(mm2.ins, guard.ins, sync=False, reason='g')
        mean = rows.tile([1, BS], F32, tag='mean', name='mean')
        var = rows.tile([1, BS], F32, tag='var', name='var')
        m2 = rows.tile([1, BS], F32, tag='m2', name='m2')
        nc.vector.tensor_scalar_mul(mean[:], ps_s[:], 1.0 / D)
        nc.vector.tensor_scalar_mul(var[:], ps_q[:], 1.0 / D)
        nc.vector.tensor_mul(m2[:], mean[:], mean[:])
        nc.vector.tensor_sub(var[:], var[:], m2[:])
        nc.vector.tensor_scalar_add(var[:], var[:], 1e-5)
        # rstd = exp(-0.5 * ln(var + eps))
        lnv = rows.tile([1, BS], F32, tag='lnv', name='lnv')
        nc.scalar.activation(lnv[:], var[:], AF.Ln)
        rstd = rows.tile([1, BS], F32R, tag='rstd', name='rstd')
        nc.scalar.activation(rstd[:], lnv[:], AF.Exp, scale=-0.5)
        ms = rows.tile([1, BS], F32R, tag='ms', name='ms')
        nc.vector.tensor_mul(ms[:], mean[:], rstd[:])
        # broadcast rows across partitions via K=1 ones-matmul into PSUM
        s_b = pss.tile([128, BS], F32, tag='st0', name='s_b')
        ms_b = pss.tile([128, BS], F32, tag='st1', name='ms_b')
        guard2 = pe_guard([rstd[:], ms[:], ones_row[:]])
        mmb = nc.tensor.matmul(s_b[:], ones_row[:], rstd[:], start=True, stop=True)
        _add_dep_helper(mmb.ins, guard2.ins, sync=False, reason='g2')
        mmb2 = nc.tensor.matmul(ms_b[:], ones_row[:], ms[:], start=True, stop=True)
        _add_dep_helper(mmb2.ins, guard2.ins, sync=False, reason='g2')
        return s_b, ms_b

    # persistent cross-block state
    Ucol = [colp.tile([128, 1], F32, tag=f'uc{i}', name=f'uc{i}') for i in range(NT)]
    U2col = [colp.tile([128, 1], F32, tag=f'u2c{i}', name=f'u2c{i}') for i in range(NT)]
    Acol = [colp.tile([128, 1], F32, tag=f'acl{i}', name=f'acl{i}') for i in range(NT)]
    Bcol = [colp.tile([128, 1], F32, tag=f'bcl{i}', name=f'bcl{i}') for i in range(NT)]
    for i in range(NT):
        nc.vector.memset(Ucol[i][:], 0.0)
        nc.vector.memset(U2col[i][:], 0.0)
        nc.vector.memset(Acol[i][:], 0.0)
        nc.vector.memset(Bcol[i][:], 0.0)

    for b in range(NB):
        t0 = b * BS
        ffn = b >= 1  # FFN & output for main blocks only

        # ---- x block + LN1 -> U ----
        xb = [ap.tile([128, BS], F32R, tag='blk', name=f'x{i}') for i in range(NT)]
        xdmas = [nc.sync.dma_start(xb[i][:], xTt[i, :, t0:t0 + BS])
                 for i in range(NT)]
        s_b, ms_b = ln_stats([xb[i][:] for i in range(NT)], xdmas, 'l1')
        U = [ap2.tile([128, BS + 1], F32, tag=f'u{i}', name=f'u{i}') for i in range(NT)]
        for i in range(NT):
            nc.vector.tensor_copy(U[i][:, 0:1], Ucol[i][:])
            t1 = scr.tile([128, BS], F32, tag='ut', name='ut')
            nc.vector.tensor_mul(t1[:], xb[i][:].bitcast(F32), s_b[:])
            nc.vector.tensor_sub(U[i][:, 1:BS + 1], t1[:], ms_b[:])
            nc.vector.tensor_copy(Ucol[i][:], U[i][:, BS:BS + 1])

        # ---- mixes (d1 shared) ----
        d1 = [ap.tile([128, BS], F32, tag='blk', name=f'd1_{i}') for i in range(NT)]
        for i in range(NT):
            nc.vector.tensor_sub(d1[i][:], U[i][:, 1:BS + 1], U[i][:, 0:BS])

        def mk_mix(mixname, tagc):
            mts = [ap.tile([128, BS], F32R, tag='blk', name=f'{tagc}{i}')
                   for i in range(NT)]
            insts = [nc.vector.scalar_tensor_tensor(
                mts[i][:], d1[i][:], cv[mixname][i],
                U[i][:, 0:BS], OP.mult, OP.add) for i in range(NT)]
            return mts, insts

        # ---- k / r / v GEMMs ----
        ink, inki = mk_mix('mixk', 'mk')
        EK = [ap.tile([128, BS], F32, tag='blk', name=f'ek{i}') for i in range(NT)]
        gemm('wk', [tl[:] for tl in ink], inki,
             lambda jt, ps: nc.scalar.activation(EK[jt][:], ps, AF.Exp,
                                                 bias=cv['bk'][jt]))
        inr, inri = mk_mix('mixr', 'mr')
        rsig = [ap.tile([128, BS], F32, tag='blk', name=f'rs{i}') for i in range(NT)]
        gemm('wr', [tl[:] for tl in inr], inri,
             lambda jt, ps: nc.scalar.activation(rsig[jt][:], ps, AF.Sigmoid,
                                                 bias=cv['br'][jt]))
        inv, invi = mk_mix('mixv', 'mv')
        vv = [ap.tile([128, BS], F32, tag='blk', name=f'vv{i}') for i in range(NT)]
        gemm('wv', [tl[:] for tl in inv], invi,
             lambda jt, ps: nc.scalar.activation(vv[jt][:], ps, AF.Identity,
                                                 bias=cv['bv'][jt]))

        # ---- WKV scans + assembly -> wkv_r ----
        wkvr = [ap.tile([128, BS], F32R, tag='blk', name=f'wr{i}') for i in range(NT)]
        wkvri = []
        for i in range(NT):
            EKV = wkvp.tile([128, BS], F32, tag='ekv', name='ekv')
            nc.vector.tensor_mul(EKV[:], EK[i][:], vv[i][:])
            A = wkvp.tile([128, BS + 1], F32, tag='A', name='A')
            B = wkvp.tile([128, BS + 1], F32, tag='B', name='B')
            if b == 1:
                # core-0 zeroes its carry here (cmask=0): the main range
                # starts from empty state like the reference's t=0
                nc.vector.tensor_mul(A[:, 0:1], Acol[i][:], cv['cmask'][i])
                nc.vector.tensor_mul(B[:, 0:1], Bcol[i][:], cv['cmask'][i])
            else:
                nc.vector.tensor_copy(A[:, 0:1], Acol[i][:])
                nc.vector.tensor_copy(B[:, 0:1], Bcol[i][:])
            ewb = cv['ew'][i].broadcast_to([128, BS])
            nc.vector.tensor_tensor_scan(A[:, 1:BS + 1], ewb, EKV[:],
                                         A[:, 0:1], OP.mult, OP.add)
            nc.vector.tensor_tensor_scan(B[:, 1:BS + 1], ewb, EK[i][:],
                                         B[:, 0:1], OP.mult, OP.add)
            nc.vector.tensor_copy(Acol[i][:], A[:, BS:BS + 1])
            nc.vector.tensor_copy(Bcol[i][:], B[:, BS:BS + 1])
            num = wkvp.tile([128, BS], F32, tag='num', name='num')
            den = wkvp.tile([128, BS], F32, tag='den', name='den')
            nc.vector.scalar_tensor_tensor(num[:], EKV[:], cv['eu'][i],
                                           A[:, 0:BS], OP.mult, OP.add)
            nc.vector.scalar_tensor_tensor(den[:], EK[i][:], cv['eu'][i],
                                           B[:, 0:BS], OP.mult, OP.add)
            rec = wkvp.tile([128, BS], F32, tag='rec', name='rec')
            nc.vector.reciprocal_approx_fast(rec[:], den[:])
            wkv = wkvp.tile([128, BS], F32, tag='wkv', name='wkv')
            nc.vector.tensor_mul(wkv[:], num[:], rec[:])
            wkvri.append(nc.vector.tensor_mul(wkvr[i][:], wkv[:], rsig[i][:]))

        # ---- atto-GEMM -> rz (x reloaded) ----
        x2 = [ap.tile([128, BS], F32, tag='blk', name=f'x2_{i}') for i in range(NT)]
        for i in range(NT):
            nc.sync.dma_start(x2[i][:], xTt[i, :, t0:t0 + BS].bitcast(F32))
        rz = [ap2.tile([128, BS], F32R, tag=f'rz{i}', name=f'rz{i}') for i in range(NT)]
        rzi = [None] * NT

        def o_cb(jt, ps):
            rzi[jt] = nc.vector.scalar_tensor_tensor(rz[jt][:], ps, cv['bo'][jt],
                                                     x2[jt][:], OP.add, OP.add)
        gemm('wo', [tl[:] for tl in wkvr], wkvri, o_cb)

        # ---- LN2 -> U2 ----
        s2b, ms2b = ln_stats([rz[i][:] for i in range(NT)], rzi, 'l2')
        U2 = [ap2.tile([128, BS + 1], F32, tag=f'w2{i}', name=f'w2{i}') for i in range(NT)]
        for i in range(NT):
            nc.vector.tensor_copy(U2[i][:, 0:1], U2col[i][:])
            t2 = scr.tile([128, BS], F32, tag='u2t', name='u2t')
            nc.vector.tensor_mul(t2[:], rz[i][:].bitcast(F32), s2b[:])
            nc.vector.tensor_sub(U2[i][:, 1:BS + 1], t2[:], ms2b[:])
            nc.vector.tensor_copy(U2col[i][:], U2[i][:, BS:BS + 1])

        if not ffn:
            continue

        # ---- FFN ----
        d2 = [ap.tile([128, BS], F32, tag='blk', name=f'e2_{i}') for i in range(NT)]
        for i in range(NT):
            nc.vector.tensor_sub(d2[i][:], U2[i][:, 1:BS + 1], U2[i][:, 0:BS])

        def mk_fmix(mixname, tagc):
            mts = [ap.tile([128, BS], F32R, tag='blk', name=f'{tagc}{i}')
                   for i in range(NT)]
            insts = [nc.vector.scalar_tensor_tensor(
                mts[i][:], d2[i][:], cv[mixname][i],
                U2[i][:, 0:BS], OP.mult, OP.add) for i in range(NT)]
            return mts, insts

        fki, fkii = mk_fmix('fmixk', 'fk')
        kf2 = [ap.tile([128, BS], F32R, tag='blk', name=f'kq{i}') for i in range(NT)]
        kf2i = [None] * NT

        def fk_cb(jt, ps):
            kf = scr.tile([128, BS], F32, tag='kf', name='kf')
            nc.scalar.activation(kf[:], ps, AF.Identity, bias=cv['bfk'][jt])
            # relu(kf)^2 == max(kf,0)*kf in one fused DVE op
            kf2i[jt] = nc.vector.scalar_tensor_tensor(kf2[jt][:], kf[:], 0.0,
                                                      kf[:], OP.max, OP.mult)
        gemm('wfk', [tl[:] for tl in fki], fkii, fk_cb)

        fri, frii = mk_fmix('fmixr', 'fr')
        rf = [ap.tile([128, BS], F32, tag='blk', name=f'rf{i}') for i in range(NT)]
        gemm('wfr', [tl[:] for tl in fri], frii,
             lambda jt, ps: nc.scalar.activation(rf[jt][:], ps, AF.Sigmoid,
                                                 bias=cv['bfr'][jt]))

        def fv_cb(jt, ps):
            t3 = scr.tile([128, BS], F32, tag='fo', name='fo')
            nc.vector.scalar_tensor_tensor(t3[:], ps, cv['bfv'][jt],
                                           rf[jt][:], OP.add, OP.mult)
            ot = scr.tile([128, BS], F32, tag='ot', name='ot')
            nc.vector.tensor_add(ot[:], t3[:], rz[jt][:].bitcast(F32))
            nc.sync.dma_start(outTt[jt, :, t0 - H:t0 - H + BS], ot[:])
        gemm('wfv', [tl[:] for tl in kf2], kf2i, fv_cb)


def prep_inputs(inputs):
    f32 = np.float32
    x = np.asarray(inputs['x'], f32)
    g1, b1 = np.asarray(inputs['ln1_g'], f32), np.asarray(inputs['ln1_b'], f32)
    g2, b2 = np.asarray(inputs['ln2_g'], f32), np.asarray(inputs['ln2_b'], f32)
    W, Bv = {}, {}
    for key, nm, g, b in [('wk', 'attk', g1, b1), ('wv', 'attv', g1, b1),
                          ('wr', 'attr', g1, b1), ('wfk', 'ffnk', g2, b2),
                          ('wfr', 'ffnr', g2, b2)]:
        w = np.asarray(inputs[nm + '_w'], f32)
        W[key] = np.ascontiguousarray((w * g[None, :]).T)
        Bv[key] = (np.asarray(inputs[nm + '_b'], f32) + w @ b).astype(f32)
    for key, nm in [('wo', 'atto'), ('wfv', 'ffnv')]:
        w = np.asarray(inputs[nm + '_w'], f32)
        W[key] = np.ascontiguousarray(w.T)
        Bv[key] = np.asarray(inputs[nm + '_b'], f32)
    bmap = dict(zip(BNAMES, ['wk', 'wv', 'wr', 'wo', 'wfk', 'wfv', 'wfr']))
    col = lambda a: np.ascontiguousarray(np.asarray(a, f32).reshape(D, 1))
    mixes = {'mixk': inputs['attmixk'], 'mixv': inputs['attmixv'],
             'mixr': inputs['attmixr'], 'fmixk': inputs['ffnmixk'],
             'fmixr': inputs['ffnmixr']}
    ew = np.exp(-np.exp(np.asarray(inputs['time_decay'], f32))).astype(f32)
    eu = np.exp(np.asarray(inputs['time_first'], f32)).astype(f32)
    xt = np.ascontiguousarray(x.T)

    in_maps = []
    for c in range(NCORES):
        s = c * TLOC
        idx = (np.arange(s - H, s + TLOC)) % T
        m = {'xT': np.ascontiguousarray(xt[:, idx])}
        for k in WNAMES:
            m[k] = W[k]
        for k in BNAMES:
            m[k] = col(Bv[bmap[k]])
        for k, v in mixes.items():
            m[k] = col(v)
        m['onescol'] = np.ones((128, 1), f32)
        m['onesrow'] = np.ones((1, 128), f32)
        m['ew'] = col(ew)
        m['eu'] = col(eu)
        m['cmask'] = np.full((D, 1), 0.0 if c == 0 else 1.0, f32)
        in_maps.append(m)
    return in_maps


_CACHED = {}
TRACE = False
LAST = {}


def kernel(**inputs):
    if 'nc' not in _CACHED:
        _CACHED['nc'] = build_kernel()
    nc = _CACHED['nc']
    in_maps = prep_inputs(inputs)
    kw = {}
    if TRACE:
        kw = dict(trace=True, trace_cores=list(range(NCORES)))
    res = run_bass_kernel_spmd(nc, in_maps, list(range(NCORES)), **kw)
    LAST['res'] = res
    parts = [np.asarray(res.results[c]['outT']) for c in range(NCORES)]
    out = np.concatenate(parts, axis=1).T
    return np.ascontiguousarray(out.astype(np.float32))


if __name__ == '__main__':
    import reference
    inputs = {k: np.asarray(v) for k, v in reference.setup_inputs().items()}
    out = kernel(**inputs)
    print('out', out.shape, out.dtype)



# revision 2
# speedup vs baseline: 1.0007x; 1.0007x over previous
"""RWKV block (T=8192, D=2048) on 8 Trainium2 NeuronCores — phase-resident v2.

Data-parallel over the sequence: 1024 main tokens/core + a 32-token recomputed
warmup prefix (power-decay |w| >= 0.65 -> state error ~e^-21, far below fp32
noise) + 1 halo column for the token shift.  Unlike the per-256-token-block
baseline (which re-streamed all 7 weight matrices from HBM for every block,
537 MB/core), v2 is phase-structured: each GEMM processes ALL of this core's
tokens while its weight streams through SBUF exactly once (59 MB/core at
bf16), keeping the tensor engine the bottleneck instead of DMA.

Layout is feature-major [D partitions, tokens free].  LayerNorm stats are
ones-matmuls; GEMMs run in bf16 (quantizing GEMM inputs injects ~0.2%-of-
sigma error; fp32r would need 2x the SBUF for resident activations); the WKV
recurrence is two fp32 tensor_tensor_scans fused per 128-channel tile into
the k/v/r phase so vector-engine work overlaps the matmuls.  Activations for
the whole 1056-token range live in a 64-slot SBUF arena of [128,1057]-bf16
tiles whose slot-reuse order (U->WKVR->RF, MK->RZ, MV->U2->KF2, MR->FMR->FMK)
matches producer/consumer retirement.  Weights are host-packed into panel
layout [j, p, kt, m] so each per-j panel DMA is a single contiguous 512 KB
read.  Core 0's warmup is the wrapped tail x[T-32:] and a cmask input zeroes
the scan carry at the warmup/main boundary so its main range starts from
empty state like the reference.
"""
import sys
if '/opt/trn_rl_repo' not in sys.path:
    sys.path.insert(0, '/opt/trn_rl_repo')

from contextlib import ExitStack
import numpy as np
import ml_dtypes

import concourse.bass as bass
import concourse.tile as tile
from concourse import bacc, mybir
from concourse.bass import _add_dep_helper
from concourse.bass_utils import run_bass_kernel_spmd

F32 = mybir.dt.float32
F32R = mybir.dt.float32r
BF16 = mybir.dt.bfloat16
AF = mybir.ActivationFunctionType
OP = mybir.AluOpType

D = 2048
T = 8192
NCORES = 8
TLOC = T // NCORES          # 1024 main tokens per core
H = 32                      # recomputed warmup tokens
NTOK = H + TLOC             # 1056 scan positions
NCOL = NTOK + 1             # 1057 = halo col + tokens
NRZ = TLOC + 1              # 1025 = last warmup token + main tokens
NT = D // 128               # 16 partition tiles

CH_A = [(0, 353), (353, 352), (705, 352)]      # over NCOL
CH_C = [(0, 352), (352, 352), (704, 352)]      # over NTOK
CH_D = [(0, 352), (352, 352), (704, 321)]      # over NRZ
CH_F = [(0, 512), (512, 512)]                  # over TLOC

WNAMES = ['wk', 'wv', 'wr', 'wo', 'wfk', 'wfr', 'wfv']
BNAMES = ['bk', 'bv', 'br', 'bo', 'bfk', 'bfr', 'bfv']
VNAMES = ['mixk', 'mixv', 'mixr', 'fmixk', 'fmixr', 'ew', 'eu', 'cmask']


def build_kernel(do_compile=True):
    nc = bacc.Bacc()
    xT = nc.declare_dram_parameter('xT', [D, NCOL], BF16, isOutput=False)
    onescol_bf = nc.declare_dram_parameter('onescol_bf', [128, 1], BF16,
                                           isOutput=False)
    onesrow = nc.declare_dram_parameter('onesrow', [1, 128], BF16, isOutput=False)
    wd = {n: nc.declare_dram_parameter(n, [D, D], BF16, isOutput=False)
          for n in WNAMES}
    vd = {n: nc.declare_dram_parameter(n, [D, 1], F32, isOutput=False)
          for n in BNAMES + VNAMES}
    outT = nc.declare_dram_parameter('outT', [D, TLOC], F32, isOutput=True)

    xTt = xT.rearrange('(n p) t -> n p t', p=128)
    outTt = outT.rearrange('(n p) t -> n p t', p=128)
    # weight panels: row j*128+p holds, for output tile j, the d_in%128==p row
    # of all 16 kt-blocks -> per-j panel DMA is one contiguous [128, 2048] read
    wdt = {n: w.rearrange('(j p) f -> j p f', p=128) for n, w in wd.items()}
    vdt = {n: v.rearrange('(n p) o -> p (n o)', p=128) for n, v in vd.items()}

    with tile.TileContext(nc) as tc:
        with ExitStack() as ctx:
            kern(ctx, tc, xTt, wdt, vdt, outTt, onescol_bf, onesrow)
    if do_compile:
        nc.compile()
    return nc


def kern(ctx, tc, xTt, wdt, vdt, outTt, onescol_bf, onesrow):
    nc = tc.nc

    cons = ctx.enter_context(tc.tile_pool(name='cons', bufs=1))
    cv = {}
    for n in BNAMES + VNAMES:
        cvt = cons.tile([128, NT], F32, tag=f'cv_{n}', name=f'cv_{n}')
        nc.sync.dma_start(cvt[:], vdt[n])
        cv[n] = [cvt[:, i:i + 1] for i in range(NT)]
    ones_bf = cons.tile([128, 1], BF16, tag='ones_bf', name='ones_bf')
    nc.sync.dma_start(ones_bf[:], onescol_bf[:])
    ones_row = cons.tile([1, 128], BF16, tag='ones_row', name='ones_row')
    nc.sync.dma_start(ones_row[:], onesrow[:])

    # activation arena: 4 groups x 16 slots of [128, NCOL] bf16; each group's
    # slots are reused in an order where the reuser's first write lands after
    # the prior tenant's last read:
    #   ga: U -> WKVR -> RF    gb: MK -> RZ    gc: MV -> U2 -> KF2
    #   gd: MR -> FMR -> FMK
    arena = ctx.enter_context(tc.tile_pool(name='arena', bufs=16))

    def atile(grp, name):
        return arena.tile([128, NCOL], BF16, tag=grp, name=name, bufs=16)

    # weight panel stream: [128, 16*128] bf16, one per (gemm, j), 6-deep ring
    pan = ctx.enter_context(tc.tile_pool(name='pan', bufs=6))
    rows = ctx.enter_context(tc.tile_pool(name='rows', bufs=1))
    t1p = ctx.enter_context(tc.tile_pool(name='t1p', bufs=2))

    def pe_guard(aps):
        """Fused-LDW matmuls can carry only ONE sync wait in the ISA. Emit a
        PE NoOp that *reads* the given APs: Tile assigns all cross-engine
        waits to it through the normal wait-clock path, so matmuls ordered
        behind it on the PE queue inherit the observed clocks (waits elided).
        The APs are stripped from the NoOp at lowering."""
        eng = nc.tensor
        inst = mybir.InstNoOp(
            name=nc.get_next_instruction_name(),
            text_hint='pe_guard', bass_nofuse=True,
            ins=[eng.lower_ap(a) for a in aps])
        return eng.add_instruction(inst)

    def panel_tile(wname, j):
        p = pan.tile([128, NT * 128], BF16, tag='pan', name=f'p_{wname}{j}',
                     bufs=6)
        nc.sync.dma_start(p[:], wdt[wname][j])
        return p

    def ln_phase(pctx, nm, chunks, pre, rhs_ap, sq_of, ubld, one,
                 post_kt=None):
        """LayerNorm over the partition axis via ones-matmuls, chunk-outer.
        pre(c0, ln) loads the chunk (LN1); rhs_ap(kt, c0, ln) -> stats rhs AP;
        sq_of(kt, c0, ln) emits the squared-chunk ACT op and returns its AP;
        ubld(kt, c0, ln, s_b, ms_b) consumes the broadcast rstd / mean*rstd
        PSUM rows to build the normalized output."""
        pst = pctx.enter_context(tc.tile_pool(name=f'pst{nm}', bufs=2,
                                              space='PSUM'))
        pbc = pctx.enter_context(tc.tile_pool(name=f'pbc{nm}', bufs=2,
                                              space='PSUM'))
        for c0, ln in chunks:
            if pre is not None:
                pre(c0, ln)
            ps_s = pst.tile([1, ln], F32, tag='st_s', name='st_s')
            ps_q = pst.tile([1, ln], F32, tag='st_q', name='st_q')
            sq0 = sq_of(0, c0, ln)
            guard = pe_guard([rhs_ap(kt, c0, ln) for kt in range(NT)]
                             + [sq0, one[:]])
            for kt in range(NT):
                sq = sq0 if kt == 0 else sq_of(kt, c0, ln)
                mm = nc.tensor.matmul(ps_s[:], one[:], rhs_ap(kt, c0, ln),
                                      start=(kt == 0), stop=(kt == NT - 1))
                _add_dep_helper(mm.ins, guard.ins, sync=False, reason='g')
                mm2 = nc.tensor.matmul(ps_q[:], one[:], sq,
                                       start=(kt == 0), stop=(kt == NT - 1))
                _add_dep_helper(mm2.ins, guard.ins, sync=False, reason='g')
            mean = rows.tile([1, 353], F32, tag='mean', name='mean')
            var = rows.tile([1, 353], F32, tag='var', name='var')
            tmp = rows.tile([1, 353], F32, tag='tmp', name='tmp')
            nc.vector.tensor_scalar_mul(mean[:, :ln], ps_s[:], 1.0 / D)
            nc.vector.tensor_scalar_mul(var[:, :ln], ps_q[:], 1.0 / D)
            nc.vector.tensor_mul(tmp[:, :ln], mean[:, :ln], mean[:, :ln])
            nc.vector.tensor_sub(var[:, :ln], var[:, :ln], tmp[:, :ln])
            nc.vector.tensor_scalar_add(var[:, :ln], var[:, :ln], 1e-5)
            # rstd = exp(-0.5 * ln(var + eps))
            nc.scalar.activation(tmp[:, :ln], var[:, :ln], AF.Ln)
            rstd = rows.tile([1, 353], BF16, tag='rstd', name='rstd')
            nc.scalar.activation(rstd[:, :ln], tmp[:, :ln], AF.Exp, scale=-0.5)
            ms = rows.tile([1, 353], BF16, tag='ms', name='ms')
            nc.vector.tensor_mul(ms[:, :ln], mean[:, :ln], rstd[:, :ln])
            s_b = pbc.tile([128, ln], F32, tag='bc_s', name='s_b')
            ms_b = pbc.tile([128, ln], F32, tag='bc_m', name='ms_b')
            g2 = pe_guard([rstd[:, :ln], ms[:, :ln], ones_row[:]])
            mb = nc.tensor.matmul(s_b[:], ones_row[:], rstd[:, :ln],
                                  start=True, stop=True)
            _add_dep_helper(mb.ins, g2.ins, sync=False, reason='g2')
            mb2 = nc.tensor.matmul(ms_b[:], ones_row[:], ms[:, :ln],
                                   start=True, stop=True)
            _add_dep_helper(mb2.ins, g2.ins, sync=False, reason='g2')
            last = (c0, ln) == chunks[-1]
            for kt in range(NT):
                ubld(kt, c0, ln, s_b[:], ms_b[:])
                if last and post_kt is not None:
                    post_kt(kt)

    # ---- phase A: x -> LN1 -> U (normalized xy, bf16, incl. halo col);
    # the MK mix is built per-tile inside the last chunk so the k-GEMM
    # can start as soon as possible ----
    U = [atile('ga', f'u{i}') for i in range(NT)]
    MK = [atile('gb', f'mk{i}') for i in range(NT)]
    with ExitStack() as pctx:
        xcp = pctx.enter_context(tc.tile_pool(name='xcp', bufs=2))
        sqp = pctx.enter_context(tc.tile_pool(name='sqp', bufs=2))
        d1pa = pctx.enter_context(tc.tile_pool(name='d1pa', bufs=2))
        xc = [None] * NT

        def pre(c0, ln):
            for kt in range(NT):
                xc[kt] = xcp.tile([128, 353], BF16, tag=f'xc{kt}',
                                  name=f'xc{kt}', bufs=2)
                nc.sync.dma_start(xc[kt][:, :ln], xTt[kt, :, c0:c0 + ln])

        def rhs_ap(kt, c0, ln):
            return xc[kt][:, :ln]

        def sq_of(kt, c0, ln):
            sq = sqp.tile([128, 353], BF16, tag='sq', name='sq', bufs=2)
            nc.scalar.activation(sq[:, :ln], xc[kt][:, :ln], AF.Square)
            return sq[:, :ln]

        def ubld(kt, c0, ln, s_b, ms_b):
            t1 = t1p.tile([128, 353], F32, tag='t1', name='t1')
            nc.vector.tensor_mul(t1[:, :ln], xc[kt][:, :ln], s_b)
            nc.vector.tensor_sub(U[kt][:, c0:c0 + ln], t1[:, :ln], ms_b)

        def post_kt(kt):
            d1 = d1pa.tile([128, NTOK], BF16, tag='d1', name='d1', bufs=2)
            nc.vector.tensor_sub(d1[:], U[kt][:, 1:NTOK + 1],
                                 U[kt][:, 0:NTOK])
            nc.vector.scalar_tensor_tensor(
                MK[kt][:, 0:NTOK], d1[:], cv['mixk'][kt],
                U[kt][:, 0:NTOK], OP.mult, OP.add)
        ln_phase(pctx, 'a', CH_A, pre, rhs_ap, sq_of, ubld, ones_bf,
                 post_kt=post_kt)

    # ---- phase B: remaining token-shift mixes (overlap the k-GEMM) ----
    MV = [atile('gc', f'mv{i}') for i in range(NT)]
    MR = [atile('gd', f'mr{i}') for i in range(NT)]
    with ExitStack() as pctx:
        d1p = pctx.enter_context(tc.tile_pool(name='d1p', bufs=2))
        # d1 recomputed per mix (gpsimd) to keep only 2 ring slots resident
        for dst, mx in [(MV, 'mixv'), (MR, 'mixr')]:
            for kt in range(NT):
                d1 = d1p.tile([128, NTOK], BF16, tag='d1', name='d1', bufs=2)
                nc.vector.tensor_sub(d1[:], U[kt][:, 1:NTOK + 1],
                                     U[kt][:, 0:NTOK])
                nc.vector.scalar_tensor_tensor(
                    dst[kt][:, 0:NTOK], d1[:], cv[mx][kt],
                    U[kt][:, 0:NTOK], OP.mult, OP.add)

    # ---- phase C: fused k/v/r GEMMs + WKV scan per output tile ----
    WKVR = [atile('ga', f'wkvr{i}') for i in range(NT)]
    with ExitStack() as cctx:
        scp = cctx.enter_context(tc.tile_pool(name='scp', bufs=1))
        psg = cctx.enter_context(tc.tile_pool(name='psgc', bufs=6,
                                              space='PSUM'))

        def cgemm(panel, rhs, guard, out_cb):
            for c0, ln in CH_C:
                ps = psg.tile([128, 352], F32, tag='ps', name='ps', bufs=6)
                for kt in range(NT):
                    mm = nc.tensor.matmul(
                        ps[:, :ln], panel[:, kt * 128:(kt + 1) * 128],
                        rhs[kt][:, c0:c0 + ln],
                        start=(kt == 0), stop=(kt == NT - 1))
                    _add_dep_helper(mm.ins, guard.ins, sync=False,
                                    reason='g')
                out_cb(c0, ln, ps[:, :ln])

        for j in range(NT):
            pk = panel_tile('wk', j)
            pv = panel_tile('wv', j)
            pr = panel_tile('wr', j)
            guard = pe_guard(
                [pk[:], pv[:], pr[:]] + [t[:] for t in MK]
                + [t[:] for t in MV] + [t[:] for t in MR])
            ek = scp.tile([128, NTOK], F32, tag='ek', name='ek', bufs=1)
            cgemm(pk, MK, guard,
                  lambda c0, ln, ps: nc.scalar.activation(
                      ek[:, c0:c0 + ln], ps, AF.Exp, bias=cv['bk'][j]))
            vv = scp.tile([128, NTOK], F32, tag='vv', name='vv', bufs=1)
            cgemm(pv, MV, guard,
                  lambda c0, ln, ps: nc.scalar.activation(
                      vv[:, c0:c0 + ln], ps, AF.Identity,
                      bias=cv['bv'][j]))
            ekv = scp.tile([128, NTOK], F32, tag='ekv', name='ekv', bufs=1)
            nc.vector.tensor_mul(ekv[:], ek[:], vv[:])
            rsig = scp.tile([128, NTOK], BF16, tag='rsig', name='rsig',
                            bufs=1)
            cgemm(pr, MR, guard,
                  lambda c0, ln, ps: nc.scalar.activation(
                      rsig[:, c0:c0 + ln], ps, AF.Sigmoid,
                      bias=cv['br'][j]))

            # A[1+s] = state after token s; A[:,H] masked so core 0's
            # main range starts from empty state like the reference
            ewb_w = cv['ew'][j].broadcast_to([128, H])
            ewb_m = cv['ew'][j].broadcast_to([128, TLOC])
            A = scp.tile([128, NCOL], F32, tag='sca', name='sca', bufs=1)
            B = scp.tile([128, NCOL], F32, tag='scb', name='scb', bufs=1)
            nc.vector.tensor_tensor_scan(A[:, 1:H + 1], ewb_w,
                                         ekv[:, 0:H], 0.0,
                                         OP.mult, OP.add)
            nc.vector.tensor_mul(A[:, H:H + 1], A[:, H:H + 1],
                                 cv['cmask'][j])
            nc.vector.tensor_tensor_scan(A[:, H + 1:NCOL], ewb_m,
                                         ekv[:, H:NTOK], A[:, H:H + 1],
                                         OP.mult, OP.add)
            nc.vector.tensor_tensor_scan(B[:, 1:H + 1], ewb_w,
                                         ek[:, 0:H], 0.0,
                                         OP.mult, OP.add)
            nc.vector.tensor_mul(B[:, H:H + 1], B[:, H:H + 1],
                                 cv['cmask'][j])
            nc.vector.tensor_tensor_scan(B[:, H + 1:NCOL], ewb_m,
                                         ek[:, H:NTOK], B[:, H:H + 1],
                                         OP.mult, OP.add)
            # wkv_s = (A_{s-1} + e^u ekv_s)/(B_{s-1} + e^u ek_s),
            # s in [H-1, NTOK); gate by r
            lo, hi = H - 1, NTOK
            num = scp.tile([128, NRZ], F32, tag='num', name='num', bufs=1)
            den = scp.tile([128, NRZ], F32, tag='den', name='den', bufs=1)
            nc.vector.scalar_tensor_tensor(num[:], ekv[:, lo:hi],
                                           cv['eu'][j], A[:, lo:hi],
                                           OP.mult, OP.add)
            nc.vector.scalar_tensor_tensor(den[:], ek[:, lo:hi],
                                           cv['eu'][j], B[:, lo:hi],
                                           OP.mult, OP.add)
            nc.vector.reciprocal_approx_fast(den[:], den[:])
            nc.vector.tensor_mul(num[:], num[:], den[:])
            nc.vector.tensor_mul(WKVR[j][:, 0:NRZ], num[:],
                                 rsig[:, lo:hi])

    # ---- phase D: atto GEMM -> rz (bf16; tokens H-1 .. NTOK) ----
    RZ = [atile('gb', f'rz{i}') for i in range(NT)]
    with ExitStack() as pctx:
        xjp = pctx.enter_context(tc.tile_pool(name='xjp', bufs=2))
        psg = pctx.enter_context(tc.tile_pool(name='psgd', bufs=6,
                                              space='PSUM'))
        for j in range(NT):
            po = panel_tile('wo', j)
            xj = xjp.tile([128, NRZ], BF16, tag='xj', name='xj', bufs=2)
            nc.sync.dma_start(xj[:], xTt[j, :, H:NCOL])
            guard = pe_guard([po[:]] + [t[:] for t in WKVR])
            for c0, ln in CH_D:
                ps = psg.tile([128, 352], F32, tag='ps', name='ps', bufs=6)
                for kt in range(NT):
                    mm = nc.tensor.matmul(
                        ps[:, :ln], po[:, kt * 128:(kt + 1) * 128],
                        WKVR[kt][:, c0:c0 + ln],
                        start=(kt == 0), stop=(kt == NT - 1))
                    _add_dep_helper(mm.ins, guard.ins, sync=False, reason='g')
                nc.vector.scalar_tensor_tensor(
                    RZ[j][:, c0:c0 + ln], ps[:, :ln], cv['bo'][j],
                    xj[:, c0:c0 + ln], OP.add, OP.add)

    # ---- phases E..G: LN2 -> U2, both FFN mixes, fr- and fk-GEMMs.
    # FMR is built per-tile inside LN2's last chunk; FMK lives in its own
    # pool and is built right after FMR, so the fk-GEMM starts with no
    # mix-build bubble after the fr-GEMM drains. ----
    U2 = [atile('gc', f'u2_{i}') for i in range(NT)]
    FMR = [atile('gd', f'fmr{i}') for i in range(NT)]
    RF = [atile('ga', f'rf{i}') for i in range(NT)]
    KF2 = [atile('gc', f'kf2_{i}') for i in range(NT)]
    with ExitStack() as fctx:
        fmkp = fctx.enter_context(tc.tile_pool(name='fmkp', bufs=1))
        FMK = [fmkp.tile([128, TLOC], BF16, tag=f'fmk{i}', name=f'fmk{i}')
               for i in range(NT)]
        with ExitStack() as pctx:
            sqbp = pctx.enter_context(tc.tile_pool(name='sqbp', bufs=2))
            d2p = pctx.enter_context(tc.tile_pool(name='d2p', bufs=2))

            def rhs_ap(kt, c0, ln):
                return RZ[kt][:, c0:c0 + ln]

            def sq_of(kt, c0, ln):
                sq = sqbp.tile([128, 352], BF16, tag='sqb', name='sqb',
                               bufs=2)
                nc.scalar.activation(sq[:, :ln], RZ[kt][:, c0:c0 + ln],
                                     AF.Square)
                return sq[:, :ln]

            def ubld(kt, c0, ln, s_b, ms_b):
                t1 = t1p.tile([128, 353], F32, tag='t1', name='t1')
                nc.vector.tensor_mul(t1[:, :ln], RZ[kt][:, c0:c0 + ln], s_b)
                nc.vector.tensor_sub(U2[kt][:, c0:c0 + ln], t1[:, :ln], ms_b)

            def post_kt(kt):
                d2 = d2p.tile([128, TLOC], BF16, tag='d2', name='d2', bufs=2)
                nc.vector.tensor_sub(d2[:], U2[kt][:, 1:NRZ],
                                     U2[kt][:, 0:TLOC])
                nc.vector.scalar_tensor_tensor(
                    FMR[kt][:, 0:TLOC], d2[:], cv['fmixr'][kt],
                    U2[kt][:, 0:TLOC], OP.mult, OP.add)
                nc.vector.scalar_tensor_tensor(
                    FMK[kt][:], d2[:], cv['fmixk'][kt],
                    U2[kt][:, 0:TLOC], OP.mult, OP.add)

            ln_phase(pctx, 'e', CH_D, None, rhs_ap, sq_of, ubld, ones_bf,
                     post_kt=post_kt)

        gemm_out(tc, nc, pan, panel_tile, pe_guard, 'wfr', FMR,
                 lambda j, c0, ln, ps: nc.scalar.activation(
                     RF[j][:, c0:c0 + ln], ps, AF.Sigmoid, bias=cv['bfr'][j]))

        with ExitStack() as pctx:
            kfp = pctx.enter_context(tc.tile_pool(name='kfp', bufs=2))

            def kf_cb(j, c0, ln, ps):
                kf = kfp.tile([128, 512], BF16, tag='kf', name='kf', bufs=2)
                nc.scalar.activation(kf[:, :ln], ps, AF.Relu,
                                     bias=cv['bfk'][j])
                nc.vector.tensor_mul(KF2[j][:, c0:c0 + ln], kf[:, :ln],
                                     kf[:, :ln])
            gemm_out(tc, nc, pan, panel_tile, pe_guard, 'wfk', FMK, kf_cb)

    # ---- phase H: FFN-v GEMM, gate by rf, add rz, store ----
    with ExitStack() as pctx:
        t3p = pctx.enter_context(tc.tile_pool(name='t3p', bufs=2))
        otp = pctx.enter_context(tc.tile_pool(name='otp', bufs=2))

        def fv_cb(j, c0, ln, ps):
            t3 = t3p.tile([128, 512], F32, tag='t3', name='t3', bufs=2)
            nc.vector.scalar_tensor_tensor(t3[:, :ln], ps, cv['bfv'][j],
                                           RF[j][:, c0:c0 + ln],
                                           OP.add, OP.mult)
            ot = otp.tile([128, 512], F32, tag='ot', name='ot', bufs=2)
            nc.vector.tensor_add(ot[:, :ln], t3[:, :ln],
                                 RZ[j][:, 1 + c0:1 + c0 + ln])
            nc.sync.dma_start(outTt[j, :, c0:c0 + ln], ot[:, :ln])
        gemm_out(tc, nc, pan, panel_tile, pe_guard, 'wfv', KF2, fv_cb)


def gemm_out(tc, nc, pan, panel_tile, pe_guard, wname, rhs, out_cb):
    """Standalone GEMM phase over the 1024 main tokens, weight streamed once."""
    with ExitStack() as pctx:
        psg = pctx.enter_context(tc.tile_pool(name=f'psg_{wname}', bufs=4,
                                              space='PSUM'))
        for j in range(NT):
            panel = panel_tile(wname, j)
            guard = pe_guard([panel[:]] + [t[:] for t in rhs])
            for c0, ln in CH_F:
                ps = psg.tile([128, 512], F32, tag='ps', name='ps', bufs=4)
                for kt in range(NT):
                    mm = nc.tensor.matmul(
                        ps[:, :ln], panel[:, kt * 128:(kt + 1) * 128],
                        rhs[kt][:, c0:c0 + ln],
                        start=(kt == 0), stop=(kt == NT - 1))
                    _add_dep_helper(mm.ins, guard.ins, sync=False, reason='g')
                out_cb(j, c0, ln, ps[:, :ln])


def prep_inputs(inputs):
    f32 = np.float32
    bf16 = ml_dtypes.bfloat16
    x = np.asarray(inputs['x'], f32)
    g1, b1 = np.asarray(inputs['ln1_g'], f32), np.asarray(inputs['ln1_b'], f32)
    g2, b2 = np.asarray(inputs['ln2_g'], f32), np.asarray(inputs['ln2_b'], f32)

    def panels(lhsT):
        # [d_in, d_out] -> [(j p), (kt m)]
        return np.ascontiguousarray(
            lhsT.reshape(16, 128, 16, 128).transpose(2, 1, 0, 3)
            .reshape(D, D).astype(bf16))

    W, Bv = {}, {}
    for key, nm, g, b in [('wk', 'attk', g1, b1), ('wv', 'attv', g1, b1),
                          ('wr', 'attr', g1, b1), ('wfk', 'ffnk', g2, b2),
                          ('wfr', 'ffnr', g2, b2)]:
        w = np.asarray(inputs[nm + '_w'], f32)
        W[key] = panels((w * g[None, :]).T)
        Bv[key] = (np.asarray(inputs[nm + '_b'], f32) + w @ b).astype(f32)
    for key, nm in [('wo', 'atto'), ('wfv', 'ffnv')]:
        w = np.asarray(inputs[nm + '_w'], f32)
        W[key] = panels(np.ascontiguousarray(w.T))
        Bv[key] = np.asarray(inputs[nm + '_b'], f32)
    bmap = dict(zip(BNAMES, ['wk', 'wv', 'wr', 'wo', 'wfk', 'wfr', 'wfv']))
    col = lambda a: np.ascontiguousarray(np.asarray(a, f32).reshape(D, 1))
    mixes = {'mixk': inputs['attmixk'], 'mixv': inputs['attmixv'],
             'mixr': inputs['attmixr'], 'fmixk': inputs['ffnmixk'],
             'fmixr': inputs['ffnmixr']}
    ew = np.exp(-np.exp(np.asarray(inputs['time_decay'], f32))).astype(f32)
    eu = np.exp(np.asarray(inputs['time_first'], f32)).astype(f32)
    xt = np.ascontiguousarray(x.T)

    in_maps = []
    for c in range(NCORES):
        s = c * TLOC
        idx = (np.arange(s - H - 1, s + TLOC)) % T
        m = {'xT': np.ascontiguousarray(xt[:, idx]).astype(bf16)}
        for k in WNAMES:
            m[k] = W[k]
        for k in BNAMES:
            m[k] = col(Bv[bmap[k]])
        for k, v in mixes.items():
            m[k] = col(v)
        m['onescol_bf'] = np.ones((128, 1), bf16)
        m['onesrow'] = np.ones((1, 128), bf16)
        m['ew'] = col(ew)
        m['eu'] = col(eu)
        m['cmask'] = np.full((D, 1), 0.0 if c == 0 else 1.0, f32)
        in_maps.append(m)
    return in_maps


_CACHED = {}
TRACE = False
LAST = {}


def kernel(**inputs):
    if 'nc' not in _CACHED:
        _CACHED['nc'] = build_kernel()
    nc = _CACHED['nc']
    in_maps = prep_inputs(inputs)
    kw = {}
    if TRACE:
        kw = dict(trace=True, trace_cores=list(range(NCORES)))
    res = run_bass_kernel_spmd(nc, in_maps, list(range(NCORES)), **kw)
    LAST['res'] = res
    parts = [np.asarray(res.results[c]['outT']) for c in range(NCORES)]
    out = np.concatenate(parts, axis=1).T
    return np.ascontiguousarray(out.astype(np.float32))


if __name__ == '__main__':
    import reference
    inputs = {k: np.asarray(v) for k, v in reference.setup_inputs().items()}
    out = kernel(**inputs)
    print('out', out.shape, out.dtype)


# revision 3
# speedup vs baseline: 1.0625x; 1.0618x over previous
"""RWKV block (T=8192, D=2048) on 8 Trainium2 NeuronCores — phase-resident v2.

Data-parallel over the sequence: 1024 main tokens/core + a 32-token recomputed
warmup prefix (power-decay |w| >= 0.65 -> state error ~e^-21, far below fp32
noise) + 1 halo column for the token shift.  Unlike the per-256-token-block
baseline (which re-streamed all 7 weight matrices from HBM for every block,
537 MB/core), v2 is phase-structured: each GEMM processes ALL of this core's
tokens while its weight streams through SBUF exactly once (59 MB/core at
bf16), keeping the tensor engine the bottleneck instead of DMA.

Layout is feature-major [D partitions, tokens free].  LayerNorm stats are
ones-matmuls; GEMMs run in bf16 (quantizing GEMM inputs injects ~0.2%-of-
sigma error; fp32r would need 2x the SBUF for resident activations); the WKV
recurrence is two fp32 tensor_tensor_scans fused per 128-channel tile into
the k/v/r phase so vector-engine work overlaps the matmuls.  Activations for
the whole 1056-token range live in a 64-slot SBUF arena of [128,1057]-bf16
tiles whose slot-reuse order (U->WKVR->RF, MK->RZ, MV->U2->KF2, MR->FMR->FMK)
matches producer/consumer retirement.  Weights are host-packed into panel
layout [j, p, kt, m] so each per-j panel DMA is a single contiguous 512 KB
read.  Core 0's warmup is the wrapped tail x[T-32:] and a cmask input zeroes
the scan carry at the warmup/main boundary so its main range starts from
empty state like the reference.
"""
import sys
if '/opt/trn_rl_repo' not in sys.path:
    sys.path.insert(0, '/opt/trn_rl_repo')

from contextlib import ExitStack
import numpy as np
import ml_dtypes

import concourse.bass as bass
import concourse.tile as tile
from concourse import bacc, mybir
from concourse.bass import _add_dep_helper
from concourse.bass_utils import run_bass_kernel_spmd

F32 = mybir.dt.float32
F32R = mybir.dt.float32r
BF16 = mybir.dt.bfloat16
AF = mybir.ActivationFunctionType
OP = mybir.AluOpType

D = 2048
T = 8192
NCORES = 8
TLOC = T // NCORES          # 1024 main tokens per core
H = 32                      # recomputed warmup tokens
NTOK = H + TLOC             # 1056 scan positions
NCOL = NTOK + 1             # 1057 = halo col + tokens
NRZ = TLOC + 1              # 1025 = last warmup token + main tokens
NT = D // 128               # 16 partition tiles

CH_A = [(0, 353), (353, 352), (705, 352)]      # over NCOL
CH_C = [(0, 352), (352, 352), (704, 352)]      # over NTOK
CH_D = [(0, 352), (352, 352), (704, 321)]      # over NRZ
CH_F = [(0, 512), (512, 512)]                  # over TLOC

WNAMES = ['wk', 'wv', 'wr', 'wo', 'wfk', 'wfr', 'wfv']
BNAMES = ['bk', 'bv', 'br', 'bo', 'bfk', 'bfr', 'bfv']
VNAMES = ['mixk', 'mixv', 'mixr', 'fmixk', 'fmixr', 'ew', 'eu', 'cmask']


def build_kernel(do_compile=True):
    nc = bacc.Bacc()
    xT = nc.declare_dram_parameter('xT', [D, NCOL], BF16, isOutput=False)
    onescol_bf = nc.declare_dram_parameter('onescol_bf', [128, 1], BF16,
                                           isOutput=False)
    onesrow = nc.declare_dram_parameter('onesrow', [1, 128], BF16, isOutput=False)
    wd = {n: nc.declare_dram_parameter(n, [D, D], BF16, isOutput=False)
          for n in WNAMES}
    vd = {n: nc.declare_dram_parameter(n, [D, 1], F32, isOutput=False)
          for n in BNAMES + VNAMES}
    outT = nc.declare_dram_parameter('outT', [D, TLOC], BF16, isOutput=True)

    xTt = xT.rearrange('(n p) t -> n p t', p=128)
    outTt = outT.rearrange('(n p) t -> n p t', p=128)
    # weight panels: row j*128+p holds, for output tile j, the d_in%128==p row
    # of all 16 kt-blocks -> per-j panel DMA is one contiguous [128, 2048] read
    wdt = {n: w.rearrange('(j p) f -> j p f', p=128) for n, w in wd.items()}
    vdt = {n: v.rearrange('(n p) o -> p (n o)', p=128) for n, v in vd.items()}

    with tile.TileContext(nc) as tc:
        with ExitStack() as ctx:
            kern(ctx, tc, xTt, wdt, vdt, outTt, onescol_bf, onesrow)
    if do_compile:
        nc.compile()
    return nc


def kern(ctx, tc, xTt, wdt, vdt, outTt, onescol_bf, onesrow):
    nc = tc.nc

    cons = ctx.enter_context(tc.tile_pool(name='cons', bufs=1))
    cv = {}
    for n in BNAMES + VNAMES:
        cvt = cons.tile([128, NT], F32, tag=f'cv_{n}', name=f'cv_{n}')
        nc.sync.dma_start(cvt[:], vdt[n])
        cv[n] = [cvt[:, i:i + 1] for i in range(NT)]
    ones_bf = cons.tile([128, 1], BF16, tag='ones_bf', name='ones_bf')
    nc.sync.dma_start(ones_bf[:], onescol_bf[:])
    ones_row = cons.tile([1, 128], BF16, tag='ones_row', name='ones_row')
    nc.sync.dma_start(ones_row[:], onesrow[:])

    # activation arena: 4 groups x 16 slots of [128, NCOL] bf16; each group's
    # slots are reused in an order where the reuser's first write lands after
    # the prior tenant's last read:
    #   ga: U -> WKVR -> RF    gb: MK -> RZ    gc: MV -> U2 -> KF2
    #   gd: MR -> FMR -> FMK
    arena = ctx.enter_context(tc.tile_pool(name='arena', bufs=16))

    def atile(grp, name):
        return arena.tile([128, NCOL], BF16, tag=grp, name=name, bufs=16)

    # weight panel stream: [128, 16*128] bf16, one per (gemm, j), 6-deep ring
    pan = ctx.enter_context(tc.tile_pool(name='pan', bufs=6))
    rows = ctx.enter_context(tc.tile_pool(name='rows', bufs=1))
    sbp = ctx.enter_context(tc.tile_pool(name='sbp', bufs=2))
    t1p = ctx.enter_context(tc.tile_pool(name='t1p', bufs=2))

    def pe_guard(aps):
        """Fused-LDW matmuls can carry only ONE sync wait in the ISA. Emit a
        PE NoOp that *reads* the given APs: Tile assigns all cross-engine
        waits to it through the normal wait-clock path, so matmuls ordered
        behind it on the PE queue inherit the observed clocks (waits elided).
        The APs are stripped from the NoOp at lowering."""
        eng = nc.tensor
        inst = mybir.InstNoOp(
            name=nc.get_next_instruction_name(),
            text_hint='pe_guard', bass_nofuse=True,
            ins=[eng.lower_ap(a) for a in aps])
        return eng.add_instruction(inst)

    def panel_tile(wname, j):
        p = pan.tile([128, NT * 128], BF16, tag='pan', name=f'p_{wname}{j}',
                     bufs=6)
        nc.sync.dma_start(p[:], wdt[wname][j])
        return p

    def ln_phase(pctx, nm, chunks, pre, rhs_ap, sq_of, ubld, one,
                 post_kt=None):
        """LayerNorm over the partition axis via ones-matmuls, chunk-outer.
        pre(c0, ln) loads the chunk (LN1); rhs_ap(kt, c0, ln) -> stats rhs AP;
        sq_of(kt, c0, ln) emits the squared-chunk ACT op and returns its AP;
        ubld(kt, c0, ln, s_b, ms_b) consumes the broadcast rstd / mean*rstd
        PSUM rows to build the normalized output."""
        pst = pctx.enter_context(tc.tile_pool(name=f'pst{nm}', bufs=2,
                                              space='PSUM'))
        pbc = pctx.enter_context(tc.tile_pool(name=f'pbc{nm}', bufs=2,
                                              space='PSUM'))
        for c0, ln in chunks:
            if pre is not None:
                pre(c0, ln)
            ps_s = pst.tile([1, ln], F32, tag='st_s', name='st_s')
            ps_q = pst.tile([1, ln], F32, tag='st_q', name='st_q')
            sq0 = sq_of(0, c0, ln)
            guard = pe_guard([rhs_ap(kt, c0, ln) for kt in range(NT)]
                             + [sq0, one[:]])
            for kt in range(NT):
                sq = sq0 if kt == 0 else sq_of(kt, c0, ln)
                mm = nc.tensor.matmul(ps_s[:], one[:], rhs_ap(kt, c0, ln),
                                      start=(kt == 0), stop=(kt == NT - 1))
                _add_dep_helper(mm.ins, guard.ins, sync=False, reason='g')
                mm2 = nc.tensor.matmul(ps_q[:], one[:], sq,
                                       start=(kt == 0), stop=(kt == NT - 1))
                _add_dep_helper(mm2.ins, guard.ins, sync=False, reason='g')
            mean = rows.tile([1, 353], F32, tag='mean', name='mean')
            var = rows.tile([1, 353], F32, tag='var', name='var')
            tmp = rows.tile([1, 353], F32, tag='tmp', name='tmp')
            nc.vector.tensor_scalar_mul(mean[:, :ln], ps_s[:], 1.0 / D)
            nc.vector.tensor_mul(tmp[:, :ln], mean[:, :ln], mean[:, :ln])
            # var = E[x^2] - mean^2 ; ln(var + eps) via ACT bias
            nc.vector.scalar_tensor_tensor(var[:, :ln], ps_q[:], 1.0 / D,
                                           tmp[:, :ln], OP.mult, OP.subtract)
            nc.vector.tensor_scalar_add(var[:, :ln], var[:, :ln], 1e-5)
            nc.scalar.activation(tmp[:, :ln], var[:, :ln], AF.Ln)
            rstd = rows.tile([1, 353], BF16, tag='rstd', name='rstd')
            nc.scalar.activation(rstd[:, :ln], tmp[:, :ln], AF.Exp, scale=-0.5)
            ms = rows.tile([1, 353], BF16, tag='ms', name='ms')
            nc.vector.tensor_mul(ms[:, :ln], mean[:, :ln], rstd[:, :ln])
            s_b = pbc.tile([128, ln], F32, tag='bc_s', name='s_b')
            ms_b = pbc.tile([128, ln], F32, tag='bc_m', name='ms_b')
            g2 = pe_guard([rstd[:, :ln], ms[:, :ln], ones_row[:]])
            mb = nc.tensor.matmul(s_b[:], ones_row[:], rstd[:, :ln],
                                  start=True, stop=True)
            _add_dep_helper(mb.ins, g2.ins, sync=False, reason='g2')
            mb2 = nc.tensor.matmul(ms_b[:], ones_row[:], ms[:, :ln],
                                   start=True, stop=True)
            _add_dep_helper(mb2.ins, g2.ins, sync=False, reason='g2')
            # stage broadcast rows to SBUF bf16 (idle ACT engine) so the
            # normalize TTs run at DVE 2x instead of 1x PSUM-src
            s_bb = sbp.tile([128, 353], BF16, tag='s_bb', name='s_bb', bufs=2)
            ms_bb = sbp.tile([128, 353], BF16, tag='ms_bb', name='ms_bb',
                             bufs=2)
            nc.scalar.activation(s_bb[:, :ln], s_b[:], AF.Copy)
            nc.scalar.activation(ms_bb[:, :ln], ms_b[:], AF.Copy)
            last = (c0, ln) == chunks[-1]
            for kt in range(NT):
                ubld(kt, c0, ln, s_bb[:, :ln], ms_bb[:, :ln])
                if last and post_kt is not None:
                    post_kt(kt)

    # ---- phase A: x -> LN1 -> U (normalized xy, bf16, incl. halo col);
    # the MK mix is built per-tile inside the last chunk so the k-GEMM
    # can start as soon as possible ----
    U = [atile('ga', f'u{i}') for i in range(NT)]
    MK = [atile('gb', f'mk{i}') for i in range(NT)]
    with ExitStack() as pctx:
        xcp = pctx.enter_context(tc.tile_pool(name='xcp', bufs=2))
        sqp = pctx.enter_context(tc.tile_pool(name='sqp', bufs=2))
        d1pa = pctx.enter_context(tc.tile_pool(name='d1pa', bufs=2))
        xc = [None] * NT

        def pre(c0, ln):
            for kt in range(NT):
                xc[kt] = xcp.tile([128, 353], BF16, tag=f'xc{kt}',
                                  name=f'xc{kt}', bufs=2)
                nc.sync.dma_start(xc[kt][:, :ln], xTt[kt, :, c0:c0 + ln])

        def rhs_ap(kt, c0, ln):
            return xc[kt][:, :ln]

        def sq_of(kt, c0, ln):
            sq = sqp.tile([128, 353], BF16, tag='sq', name='sq', bufs=2)
            nc.scalar.activation(sq[:, :ln], xc[kt][:, :ln], AF.Square)
            return sq[:, :ln]

        def ubld(kt, c0, ln, s_b, ms_b):
            t1 = t1p.tile([128, 353], BF16, tag='t1', name='t1')
            nc.vector.tensor_mul(t1[:, :ln], xc[kt][:, :ln], s_b)
            nc.vector.tensor_sub(U[kt][:, c0:c0 + ln], t1[:, :ln], ms_b)

        def post_kt(kt):
            d1 = d1pa.tile([128, NTOK], BF16, tag='d1', name='d1', bufs=2)
            nc.vector.tensor_sub(d1[:], U[kt][:, 1:NTOK + 1],
                                 U[kt][:, 0:NTOK])
            nc.vector.scalar_tensor_tensor(
                MK[kt][:, 0:NTOK], d1[:], cv['mixk'][kt],
                U[kt][:, 0:NTOK], OP.mult, OP.add)
        ln_phase(pctx, 'a', CH_A, pre, rhs_ap, sq_of, ubld, ones_bf,
                 post_kt=post_kt)

    # ---- phase B: remaining token-shift mixes (overlap the k-GEMM) ----
    MV = [atile('gc', f'mv{i}') for i in range(NT)]
    MR = [atile('gd', f'mr{i}') for i in range(NT)]
    with ExitStack() as pctx:
        d1p = pctx.enter_context(tc.tile_pool(name='d1p', bufs=2))
        # d1 recomputed per mix (gpsimd) to keep only 2 ring slots resident
        for dst, mx in [(MV, 'mixv'), (MR, 'mixr')]:
            for kt in range(NT):
                d1 = d1p.tile([128, NTOK], BF16, tag='d1', name='d1', bufs=2)
                nc.vector.tensor_sub(d1[:], U[kt][:, 1:NTOK + 1],
                                     U[kt][:, 0:NTOK])
                nc.vector.scalar_tensor_tensor(
                    dst[kt][:, 0:NTOK], d1[:], cv[mx][kt],
                    U[kt][:, 0:NTOK], OP.mult, OP.add)

    # ---- phase C: fused k/v/r GEMMs + WKV scan per output tile ----
    WKVR = [atile('ga', f'wkvr{i}') for i in range(NT)]
    with ExitStack() as cctx:
        scp = cctx.enter_context(tc.tile_pool(name='scp', bufs=1))
        psg = cctx.enter_context(tc.tile_pool(name='psgc', bufs=6,
                                              space='PSUM'))

        def cgemm(panel, rhs, guard, out_cb):
            for c0, ln in CH_C:
                ps = psg.tile([128, 352], F32, tag='ps', name='ps', bufs=6)
                for kt in range(NT):
                    mm = nc.tensor.matmul(
                        ps[:, :ln], panel[:, kt * 128:(kt + 1) * 128],
                        rhs[kt][:, c0:c0 + ln],
                        start=(kt == 0), stop=(kt == NT - 1))
                    _add_dep_helper(mm.ins, guard.ins, sync=False,
                                    reason='g')
                out_cb(c0, ln, ps[:, :ln])

        for j in range(NT):
            pk = panel_tile('wk', j)
            pv = panel_tile('wv', j)
            pr = panel_tile('wr', j)
            guard_k = pe_guard([pk[:]] + [t[:] for t in MK])
            guard_v = pe_guard([pv[:]] + [t[:] for t in MV])
            guard_r = pe_guard([pr[:]] + [t[:] for t in MR])
            ek = scp.tile([128, NTOK], F32, tag='ek', name='ek', bufs=1)
            cgemm(pk, MK, guard_k,
                  lambda c0, ln, ps: nc.scalar.activation(
                      ek[:, c0:c0 + ln], ps, AF.Exp, bias=cv['bk'][j]))
            vv = scp.tile([128, NTOK], F32, tag='vv', name='vv', bufs=1)
            cgemm(pv, MV, guard_v,
                  lambda c0, ln, ps: nc.scalar.activation(
                      vv[:, c0:c0 + ln], ps, AF.Identity,
                      bias=cv['bv'][j]))
            ekv = scp.tile([128, NTOK], F32, tag='ekv', name='ekv', bufs=1)
            nc.vector.tensor_mul(ekv[:], ek[:], vv[:])
            rsig = scp.tile([128, NTOK], BF16, tag='rsig', name='rsig',
                            bufs=1)
            cgemm(pr, MR, guard_r,
                  lambda c0, ln, ps: nc.scalar.activation(
                      rsig[:, c0:c0 + ln], ps, AF.Sigmoid,
                      bias=cv['br'][j]))

            # A[1+s] = state after token s; A[:,H] masked so core 0's
            # main range starts from empty state like the reference
            ewb_w = cv['ew'][j].broadcast_to([128, H])
            ewb_m = cv['ew'][j].broadcast_to([128, TLOC])
            A = scp.tile([128, NCOL], F32, tag='sca', name='sca', bufs=1)
            B = scp.tile([128, NCOL], F32, tag='scb', name='scb', bufs=1)
            nc.vector.tensor_tensor_scan(A[:, 1:H + 1], ewb_w,
                                         ekv[:, 0:H], 0.0,
                                         OP.mult, OP.add)
            nc.vector.tensor_mul(A[:, H:H + 1], A[:, H:H + 1],
                                 cv['cmask'][j])
            nc.vector.tensor_tensor_scan(A[:, H + 1:NCOL], ewb_m,
                                         ekv[:, H:NTOK], A[:, H:H + 1],
                                         OP.mult, OP.add)
            nc.vector.tensor_tensor_scan(B[:, 1:H + 1], ewb_w,
                                         ek[:, 0:H], 0.0,
                                         OP.mult, OP.add)
            nc.vector.tensor_mul(B[:, H:H + 1], B[:, H:H + 1],
                                 cv['cmask'][j])
            nc.vector.tensor_tensor_scan(B[:, H + 1:NCOL], ewb_m,
                                         ek[:, H:NTOK], B[:, H:H + 1],
                                         OP.mult, OP.add)
            # wkv_s = (A_{s-1} + e^u ekv_s)/(B_{s-1} + e^u ek_s),
            # s in [H-1, NTOK); gate by r
            lo, hi = H - 1, NTOK
            num = scp.tile([128, NRZ], F32, tag='num', name='num', bufs=1)
            den = scp.tile([128, NRZ], F32, tag='den', name='den', bufs=1)
            nc.vector.scalar_tensor_tensor(num[:], ekv[:, lo:hi],
                                           cv['eu'][j], A[:, lo:hi],
                                           OP.mult, OP.add)
            nc.vector.scalar_tensor_tensor(den[:], ek[:, lo:hi],
                                           cv['eu'][j], B[:, lo:hi],
                                           OP.mult, OP.add)
            nc.vector.reciprocal_approx_fast(den[:], den[:])
            nc.vector.tensor_mul(num[:], num[:], den[:])
            nc.vector.tensor_mul(WKVR[j][:, 0:NRZ], num[:],
                                 rsig[:, lo:hi])

    # ---- phase D: atto GEMM -> rz (bf16; tokens H-1 .. NTOK) ----
    RZ = [atile('gb', f'rz{i}') for i in range(NT)]
    with ExitStack() as pctx:
        xjp = pctx.enter_context(tc.tile_pool(name='xjp', bufs=2))
        psg = pctx.enter_context(tc.tile_pool(name='psgd', bufs=6,
                                              space='PSUM'))
        for j in range(NT):
            po = panel_tile('wo', j)
            xj = xjp.tile([128, NRZ], BF16, tag='xj', name='xj', bufs=2)
            nc.sync.dma_start(xj[:], xTt[j, :, H:NCOL])
            guard = pe_guard([po[:]] + [t[:] for t in WKVR])
            for c0, ln in CH_D:
                ps = psg.tile([128, 352], F32, tag='ps', name='ps', bufs=6)
                for kt in range(NT):
                    mm = nc.tensor.matmul(
                        ps[:, :ln], po[:, kt * 128:(kt + 1) * 128],
                        WKVR[kt][:, c0:c0 + ln],
                        start=(kt == 0), stop=(kt == NT - 1))
                    _add_dep_helper(mm.ins, guard.ins, sync=False, reason='g')
                nc.vector.scalar_tensor_tensor(
                    RZ[j][:, c0:c0 + ln], ps[:, :ln], cv['bo'][j],
                    xj[:, c0:c0 + ln], OP.add, OP.add)

    # ---- phases E..G: LN2 -> U2, both FFN mixes, fr- and fk-GEMMs.
    # FMR is built per-tile inside LN2's last chunk; FMK lives in its own
    # pool and is built right after FMR, so the fk-GEMM starts with no
    # mix-build bubble after the fr-GEMM drains. ----
    U2 = [atile('gc', f'u2_{i}') for i in range(NT)]
    FMR = [atile('gd', f'fmr{i}') for i in range(NT)]
    RF = [atile('ga', f'rf{i}') for i in range(NT)]
    KF2 = [atile('gc', f'kf2_{i}') for i in range(NT)]
    with ExitStack() as fctx:
        fmkp = fctx.enter_context(tc.tile_pool(name='fmkp', bufs=1))
        FMK = [fmkp.tile([128, TLOC], BF16, tag=f'fmk{i}', name=f'fmk{i}')
               for i in range(NT)]
        with ExitStack() as pctx:
            sqbp = pctx.enter_context(tc.tile_pool(name='sqbp', bufs=2))
            d2p = pctx.enter_context(tc.tile_pool(name='d2p', bufs=2))

            def rhs_ap(kt, c0, ln):
                return RZ[kt][:, c0:c0 + ln]

            def sq_of(kt, c0, ln):
                sq = sqbp.tile([128, 352], BF16, tag='sqb', name='sqb',
                               bufs=2)
                nc.scalar.activation(sq[:, :ln], RZ[kt][:, c0:c0 + ln],
                                     AF.Square)
                return sq[:, :ln]

            def ubld(kt, c0, ln, s_b, ms_b):
                t1 = t1p.tile([128, 353], BF16, tag='t1', name='t1')
                nc.vector.tensor_mul(t1[:, :ln], RZ[kt][:, c0:c0 + ln], s_b)
                nc.vector.tensor_sub(U2[kt][:, c0:c0 + ln], t1[:, :ln], ms_b)

            def post_kt(kt):
                d2 = d2p.tile([128, TLOC], BF16, tag='d2', name='d2', bufs=2)
                nc.vector.tensor_sub(d2[:], U2[kt][:, 1:NRZ],
                                     U2[kt][:, 0:TLOC])
                nc.vector.scalar_tensor_tensor(
                    FMR[kt][:, 0:TLOC], d2[:], cv['fmixr'][kt],
                    U2[kt][:, 0:TLOC], OP.mult, OP.add)

            ln_phase(pctx, 'e', CH_D, None, rhs_ap, sq_of, ubld, ones_bf,
                     post_kt=post_kt)
            for kt in range(NT):
                d2 = d2p.tile([128, TLOC], BF16, tag='d2', name='d2', bufs=2)
                nc.vector.tensor_sub(d2[:], U2[kt][:, 1:NRZ],
                                     U2[kt][:, 0:TLOC])
                nc.vector.scalar_tensor_tensor(
                    FMK[kt][:], d2[:], cv['fmixk'][kt],
                    U2[kt][:, 0:TLOC], OP.mult, OP.add)

        gemm_out(tc, nc, pan, panel_tile, pe_guard, 'wfr', FMR,
                 lambda j, c0, ln, ps: nc.scalar.activation(
                     RF[j][:, c0:c0 + ln], ps, AF.Sigmoid, bias=cv['bfr'][j]))

        with ExitStack() as pctx:
            kfp = pctx.enter_context(tc.tile_pool(name='kfp', bufs=2))

            def kf_cb(j, c0, ln, ps):
                kf = kfp.tile([128, 512], BF16, tag='kf', name='kf', bufs=2)
                nc.scalar.activation(kf[:, :ln], ps, AF.Relu,
                                     bias=cv['bfk'][j])
                nc.vector.tensor_mul(KF2[j][:, c0:c0 + ln], kf[:, :ln],
                                     kf[:, :ln])
            gemm_out(tc, nc, pan, panel_tile, pe_guard, 'wfk', FMK, kf_cb)

    # ---- phase H: FFN-v GEMM, gate by rf, add rz, store ----
    with ExitStack() as pctx:
        t3p = pctx.enter_context(tc.tile_pool(name='t3p', bufs=2))
        otp = pctx.enter_context(tc.tile_pool(name='otp', bufs=2))

        def fv_cb(j, c0, ln, ps):
            t3 = t3p.tile([128, 512], BF16, tag='t3', name='t3', bufs=2)
            nc.vector.scalar_tensor_tensor(t3[:, :ln], ps, cv['bfv'][j],
                                           RF[j][:, c0:c0 + ln],
                                           OP.add, OP.mult)
            ot = otp.tile([128, 512], BF16, tag='ot', name='ot', bufs=2)
            nc.vector.tensor_add(ot[:, :ln], t3[:, :ln],
                                 RZ[j][:, 1 + c0:1 + c0 + ln])
            nc.sync.dma_start(outTt[j, :, c0:c0 + ln], ot[:, :ln])
        gemm_out(tc, nc, pan, panel_tile, pe_guard, 'wfv', KF2, fv_cb)


def gemm_out(tc, nc, pan, panel_tile, pe_guard, wname, rhs, out_cb):
    """Standalone GEMM phase over the 1024 main tokens, weight streamed once."""
    with ExitStack() as pctx:
        psg = pctx.enter_context(tc.tile_pool(name=f'psg_{wname}', bufs=4,
                                              space='PSUM'))
        for j in range(NT):
            panel = panel_tile(wname, j)
            guard = pe_guard([panel[:]] + [t[:] for t in rhs])
            for c0, ln in CH_F:
                ps = psg.tile([128, 512], F32, tag='ps', name='ps', bufs=4)
                for kt in range(NT):
                    mm = nc.tensor.matmul(
                        ps[:, :ln], panel[:, kt * 128:(kt + 1) * 128],
                        rhs[kt][:, c0:c0 + ln],
                        start=(kt == 0), stop=(kt == NT - 1))
                    _add_dep_helper(mm.ins, guard.ins, sync=False, reason='g')
                out_cb(j, c0, ln, ps[:, :ln])


def prep_inputs(inputs):
    f32 = np.float32
    bf16 = ml_dtypes.bfloat16
    x = np.asarray(inputs['x'], f32)
    g1, b1 = np.asarray(inputs['ln1_g'], f32), np.asarray(inputs['ln1_b'], f32)
    g2, b2 = np.asarray(inputs['ln2_g'], f32), np.asarray(inputs['ln2_b'], f32)

    def panels(lhsT):
        # [d_in, d_out] -> [(j p), (kt m)]
        return np.ascontiguousarray(
            lhsT.reshape(16, 128, 16, 128).transpose(2, 1, 0, 3)
            .reshape(D, D).astype(bf16))

    W, Bv = {}, {}
    for key, nm, g, b in [('wk', 'attk', g1, b1), ('wv', 'attv', g1, b1),
                          ('wr', 'attr', g1, b1), ('wfk', 'ffnk', g2, b2),
                          ('wfr', 'ffnr', g2, b2)]:
        w = np.asarray(inputs[nm + '_w'], f32)
        W[key] = panels((w * g[None, :]).T)
        Bv[key] = (np.asarray(inputs[nm + '_b'], f32) + w @ b).astype(f32)
    for key, nm in [('wo', 'atto'), ('wfv', 'ffnv')]:
        w = np.asarray(inputs[nm + '_w'], f32)
        W[key] = panels(np.ascontiguousarray(w.T))
        Bv[key] = np.asarray(inputs[nm + '_b'], f32)
    bmap = dict(zip(BNAMES, ['wk', 'wv', 'wr', 'wo', 'wfk', 'wfr', 'wfv']))
    col = lambda a: np.ascontiguousarray(np.asarray(a, f32).reshape(D, 1))
    mixes = {'mixk': inputs['attmixk'], 'mixv': inputs['attmixv'],
             'mixr': inputs['attmixr'], 'fmixk': inputs['ffnmixk'],
             'fmixr': inputs['ffnmixr']}
    ew = np.exp(-np.exp(np.asarray(inputs['time_decay'], f32))).astype(f32)
    eu = np.exp(np.asarray(inputs['time_first'], f32)).astype(f32)
    xt = np.ascontiguousarray(x.T)

    in_maps = []
    for c in range(NCORES):
        s = c * TLOC
        idx = (np.arange(s - H - 1, s + TLOC)) % T
        m = {'xT': np.ascontiguousarray(xt[:, idx]).astype(bf16)}
        for k in WNAMES:
            m[k] = W[k]
        for k in BNAMES:
            m[k] = col(Bv[bmap[k]])
        for k, v in mixes.items():
            m[k] = col(v)
        m['onescol_bf'] = np.ones((128, 1), bf16)
        m['onesrow'] = np.ones((1, 128), bf16)
        m['ew'] = col(ew)
        m['eu'] = col(eu)
        m['cmask'] = np.full((D, 1), 0.0 if c == 0 else 1.0, f32)
        in_maps.append(m)
    return in_maps


_CACHED = {}
TRACE = False
LAST = {}


def kernel(**inputs):
    if 'nc' not in _CACHED:
        _CACHED['nc'] = build_kernel()
    nc = _CACHED['nc']
    in_maps = prep_inputs(inputs)
    kw = {}
    if TRACE:
        kw = dict(trace=True, trace_cores=list(range(NCORES)))
    res = run_bass_kernel_spmd(nc, in_maps, list(range(NCORES)), **kw)
    LAST['res'] = res
    parts = [np.asarray(res.results[c]['outT']) for c in range(NCORES)]
    out = np.concatenate(parts, axis=1).T
    return np.ascontiguousarray(out.astype(np.float32))


if __name__ == '__main__':
    import reference
    inputs = {k: np.asarray(v) for k, v in reference.setup_inputs().items()}
    out = kernel(**inputs)
    print('out', out.shape, out.dtype)
